# revision 1
# baseline (speedup 1.0000x reference)
"""Low-rank INR layer on 8 trn2 NeuronCores.

out[b] = relu((x[b] @ v[b].T) @ W.T),  x:[16,8192,512] v:[16,32,512] W:[512,32]

Sharding: data-parallel over batch b — 2 scenes per core, W replicated.

Per-core kernel (all fp32 storage, fp32r matmuls ~1.5e-4 rel err):
  for each batch b (2), n-chunk c (16 x 512 rows):
    - DMA 4 natural x tiles [128n, 512d]
    - PE-transpose (identity matmul) into [128d, 512n] tiles (x must have
      d on partitions for the contraction; fp32 has no DMA transpose)
    - stage 1: xv^T[32r, 512n] += vT[d,r].T @ xT[d,n] over 4 d-chunks
    - stage 2: out[128n, 512o] = xvT[r,n-slice].T @ wT[r,o], relu on ACT
    - DMA out natural tiles
v^T / W^T are transposed on host (tiny) and rounded to fp32r on-chip.
"""
import os
import numpy as np
from contextlib import ExitStack

import concourse.bass as bass
import concourse.tile as tile
from concourse import bacc, mybir
from concourse.bass_utils import run_bass_kernel_spmd
from concourse.masks import make_identity

B, N, DI, DO, R = 16, 8192, 512, 512, 32
NCORES = 8
BPC = B // NCORES  # batches per core
NCHUNK = 512  # n rows per chunk
NSUB = NCHUNK // 128  # 4 natural tiles per chunk
DCH = DI // 128  # 4 d-chunks

f32 = mybir.dt.float32
f32r = mybir.dt.float32r

_BUILD_CACHE = {}


def _build(repeat=1):
    if repeat in _BUILD_CACHE:
        return _BUILD_CACHE[repeat]
    nc = bacc.Bacc("TRN2", target_bir_lowering=False, debug=False, num_devices=NCORES)
    x = nc.dram_tensor("x", [BPC, N, DI], f32, kind="ExternalInput").ap()
    vT = nc.dram_tensor("vT", [BPC, DI, R], f32, kind="ExternalInput").ap()
    wT = nc.dram_tensor("wT", [R, DO], f32, kind="ExternalInput").ap()
    out = nc.dram_tensor("out", [BPC, N, DO], f32, kind="ExternalOutput").ap()

    with tile.TileContext(nc) as tc, ExitStack() as ctx:
        const = ctx.enter_context(tc.tile_pool(name="const", bufs=1))
        xn_pool = ctx.enter_context(tc.tile_pool(name="xn", bufs=8))
        xt_pool = ctx.enter_context(tc.tile_pool(name="xt", bufs=8))
        xv_pool = ctx.enter_context(tc.tile_pool(name="xv", bufs=2))
        ob_pool = ctx.enter_context(tc.tile_pool(name="ob", bufs=6))
        ps_t = ctx.enter_context(tc.tile_pool(name="ps_t", bufs=2, space="PSUM"))
        ps_xv = ctx.enter_context(tc.tile_pool(name="ps_xv", bufs=2, space="PSUM"))
        ps_o = ctx.enter_context(tc.tile_pool(name="ps_o", bufs=3, space="PSUM"))

        # constants
        ident = const.tile([128, 128], f32)
        make_identity(nc, ident[:])
        bias0 = const.tile([128, 1], f32)
        nc.vector.memset(bias0[:], 0.0)

        # v^T: SBUF [128(d-local), BPC*DCH*32(r)] fp32 -> round to fp32r
        vt_f = const.tile([128, BPC * DCH * 32], f32)
        for b in range(BPC):
            for dc in range(DCH):
                nc.sync.dma_start(
                    vt_f[:, (b * DCH + dc) * 32 : (b * DCH + dc + 1) * 32],
                    vT[b, dc * 128 : (dc + 1) * 128, :],
                )
        vt_r = const.tile([128, BPC * DCH * 32], f32r)
        nc.vector.tensor_copy(vt_r[:], vt_f[:])

        # W^T: SBUF [32(r), 512(o)] -> fp32r
        wt_f = const.tile([32, DO], f32)
        nc.sync.dma_start(wt_f[:], wT[:, :])
        wt_r = const.tile([32, DO], f32r)
        nc.vector.tensor_copy(wt_r[:], wt_f[:])

        def body(_iv=None):
            for b in range(BPC):
                for c in range(N // NCHUNK):
                    n0 = c * NCHUNK
                    # natural loads [128n, 512d]
                    xn = []
                    for s in range(NSUB):
                        t = xn_pool.tile([128, DI], f32, tag="xn")
                        nc.sync.dma_start(
                            t[:], x[b, n0 + s * 128 : n0 + (s + 1) * 128, :]
                        )
                        xn.append(t)
                    # transpose to [128d, 512n] per d-chunk, round to fp32r
                    xt = []
                    for dc in range(DCH):
                        pst = ps_t.tile([128, NCHUNK], f32, tag="pst")
                        for s in range(NSUB):
                            nc.tensor.transpose(
                                pst[:, s * 128 : (s + 1) * 128],
                                xn[s][:, dc * 128 : (dc + 1) * 128],
                                ident[:],
                            )
                        xtt = xt_pool.tile([128, NCHUNK], f32r, tag="xt")
                        nc.vector.tensor_copy(xtt[:], pst[:])
                        xt.append(xtt)
                    # stage 1: xv^T [32, 512] accumulate over d-chunks
                    psx = ps_xv.tile([32, NCHUNK], f32, tag="psx")
                    for dc in range(DCH):
                        col = (b * DCH + dc) * 32
                        nc.tensor.matmul(
                            psx[:],
                            vt_r[:, col : col + 32],
                            xt[dc][:],
                            start=(dc == 0),
                            stop=(dc == DCH - 1),
                        )
                    xvt = xv_pool.tile([32, NCHUNK], f32r, tag="xvt")
                    nc.vector.tensor_copy(xvt[:], psx[:])
                    # stage 2 + relu + store
                    for s in range(NSUB):
                        pso = ps_o.tile([128, DO], f32, tag="pso")
                        nc.tensor.matmul(
                            pso[:],
                            xvt[:, s * 128 : (s + 1) * 128],
                            wt_r[:],
                            start=True,
                            stop=True,
                        )
                        ob = ob_pool.tile([128, DO], f32, tag="ob")
                        nc.scalar.activation(
                            ob[:],
                            pso[:],
                            mybir.ActivationFunctionType.Relu,
                            bias=bias0[:],
                        )
                        nc.sync.dma_start(
                            out[b, n0 + s * 128 : n0 + (s + 1) * 128, :], ob[:]
                        )

        if repeat == 1:
            body()
        else:
            with tc.For_i(0, repeat, 1) as iv:
                body(iv)

    nc.compile()
    _BUILD_CACHE[repeat] = nc
    return nc


def _prep_in_maps(x, v_mat, W):
    x = np.ascontiguousarray(x, dtype=np.float32)
    vT_full = np.ascontiguousarray(np.transpose(v_mat, (0, 2, 1)), dtype=np.float32)
    wT = np.ascontiguousarray(W.T, dtype=np.float32)
    return [
        {
            "x": x[i * BPC : (i + 1) * BPC],
            "vT": vT_full[i * BPC : (i + 1) * BPC],
            "wT": wT,
        }
        for i in range(NCORES)
    ]


def run_sharded(x, v_mat, W, repeat=1):
    nc = _build(repeat)
    in_maps = _prep_in_maps(x, v_mat, W)
    res = run_bass_kernel_spmd(nc, in_maps, list(range(NCORES)))
    return np.concatenate([res.results[i]["out"] for i in range(NCORES)], axis=0)


def kernel(x, v_mat, W):
    return run_sharded(x, v_mat, W, repeat=1)
